# revision 1
# baseline (speedup 1.0000x reference)
"""LoRA layer kernel for Trainium2, 8-core data-parallel.

out = x @ W.T + 2.0 * ((x @ B) @ A)
  x: (4, 4096, 4096) f32, W: (4096, 4096), A: (16, 4096), B: (4096, 16)

Strategy: flatten x to (16384, 4096) rows, shard rows across 8 cores
(2048 rows each), replicate W/A/B. All matmul operands in bf16 (PSUM
accumulation stays fp32; end-to-end rel err ~2e-3 vs the 2e-2 gate).

Per core, single x-resident block (2048 rows = 128 KB/partition bf16):
  - x loads m-split into 512-col quarters so mt-quarter q + stage-A
    chunk h=q unlock after 1/4 of x lands (keeps PE fed during the
    HBM-bound load).
  - stage A (tT = (x @ B).T) 4-way column-tiled: k-tile k -> PE col
    group k%4, partials in 4 PSUM banks at partition stripes
    {32j..32j+15}; fused per-k with main quarter q=g for oc0 so the
    in-order PE queue never head-of-line blocks on x arrival.
  - LoRA fold for oc0: one K=128 matmul per mt against a2rep (2*A at
    the 4 stripes, zeros elsewhere; tT4 memset kills NaN garbage).
  - LoRA fold for oc1..7: partial stripes packed+duplicated into tTp
    rows 0-63 / 64-127 via SBUF->SBUF DMAs; accum is 2 concurrent
    row-tiled K=64 matmuls per quarter (tile_position (0,0)/(64,0)).
  - main GEMM: per o-chunk, 4 mt-quarters of 4 PSUM banks; x-tile
    stationary, W streamed as k-quads [128, 4*512] on sync.
Queues: W/consts/dups on sync, out stores alternate scalar/sync,
x loads alternate gpsimd/scalar.
"""

import sys

if "/opt/trn_rl_repo" not in sys.path:
    sys.path.insert(0, "/opt/trn_rl_repo")

import numpy as np
import ml_dtypes

import concourse.bass as bass
import concourse.mybir as mybir
import concourse.tile as tile

N_CORES = 8
D = 4096
RANK = 16
ROWS_TOTAL = 4 * 4096          # 16384
M = ROWS_TOTAL // N_CORES      # 2048 rows per core
P = 128
KT = D // P                    # 32 k-tiles
OC = 512                       # o-chunk width (one PSUM bank)
N_OC = D // OC                 # 8
MT = M // P                    # 16 m-tiles
MQ = 4                         # m-tiles per quarter (PSUM banks)
NQ = MT // MQ                  # 4 quarters
KQ = 4                         # k-tiles per W quad DMA
N_KQ = KT // KQ                # 8

F32 = mybir.dt.float32
BF16 = mybir.dt.bfloat16
BF16_NP = ml_dtypes.bfloat16

N_WARMUP = 8


def split_wide_waits(nc, max_waits=1):
    """walrus in this container rejects >1 sync wait per instruction;
    move excess waits onto preceding same-engine NoOps."""
    n_split = 0
    for f in nc.m.functions:
        for bb in f.blocks:
            new_insts = []
            for inst in bb.instructions:
                si = getattr(inst, "sync_info", None)
                if si is not None and si.on_wait and len(si.on_wait) > max_waits:
                    waits = list(si.on_wait)
                    keep = waits[-max_waits:]
                    extra = waits[:-max_waits]
                    for i in range(0, len(extra), max_waits):
                        chunk = extra[i:i + max_waits]
                        nop = mybir.InstNoOp(
                            name=f"{inst.name}_wsplit{i}",
                            sync_info=mybir.SyncInfo(on_wait=chunk, on_update=[]),
                            bass_nofuse=True,
                            engine=inst.engine,
                        )
                        new_insts.append(nop)
                        n_split += 1
                    si.on_wait = keep
                new_insts.append(inst)
            bb.instructions[:] = new_insts
    return n_split


def build_program():
    nc = bass.Bass()
    xt = nc.declare_dram_parameter("xt", [D, M], BF16, isOutput=False)
    wt = nc.declare_dram_parameter("wt", [D, D], BF16, isOutput=False)
    # bmat pre-arranged on host: [128, KT*RANK], col-block k = rows k*128..+128
    bmat = nc.declare_dram_parameter("bmat", [P, KT * RANK], BF16, isOutput=False)
    # a2rep: 2*A at partition stripes {32j..32j+15}, zeros elsewhere (K=128 fold)
    a2rep = nc.declare_dram_parameter("a2rep", [P, D], BF16, isOutput=False)
    # a2p: 2*A tiled 4x in rows 0-63 and again in rows 64-127 (K=64 pair fold)
    a2p = nc.declare_dram_parameter("a2p", [P, D], BF16, isOutput=False)
    out = nc.declare_dram_parameter("out", [M, D], F32, isOutput=True)

    with tile.TileContext(nc) as tc:
        with (
            tc.tile_pool(name="xpool", bufs=1) as xpool,
            tc.tile_pool(name="wpool", bufs=9) as wpool,
            tc.tile_pool(name="opool", bufs=2) as opool,
            tc.tile_pool(name="cpool", bufs=1) as cpool,
            tc.tile_pool(name="tpool", bufs=1) as tpool,
            tc.tile_pool(name="ppool", bufs=8, space="PSUM") as ppool,
        ):
            # constants on sync queue
            btile = cpool.tile([P, KT * RANK], BF16, tag="bt")
            nc.sync.dma_start(btile[:], bmat[:])
            atile_r = cpool.tile([P, D], BF16, tag="ar")
            nc.sync.dma_start(atile_r[:], a2rep[:])
            atile_p = cpool.tile([P, D], BF16, tag="ap")
            nc.sync.dma_start(atile_p[:], a2p[:])

            # x fully resident, laid out [k4][mq][kk][m] so each
            # k-quad x m-quarter block is flat-contiguous: one 512KB DMA
            # per block (32 total), arriving at exactly the granularity
            # the PE consumes (stage quad + 16 main MMs per block).
            xall = xpool.tile([P, KT * M], BF16, tag="x")

            def xsl(k, c0, cw):
                k4, kk = divmod(k, KQ)
                mq, d = divmod(c0, OC)
                assert d + cw <= OC
                base = ((k4 * NQ + mq) * KQ + kk) * OC + d
                return xall[:, base: base + cw]

            nx = 0
            for mq in range(NQ):
                for k4 in range(N_KQ):
                    eng = nc.gpsimd if nx % 2 == 0 else nc.scalar
                    nx += 1
                    base = (k4 * NQ + mq) * KQ * OC
                    eng.dma_start(
                        xall[:, base: base + KQ * OC].rearrange(
                            "p (b c) -> p b c", b=KQ),
                        xt[k4 * KQ * P:(k4 + 1) * KQ * P,
                           mq * OC:(mq + 1) * OC].rearrange(
                               "(b p) c -> p b c", p=P),
                    )

            # tT4 [128, M]: stage-A partials at stripes {32j..32j+15};
            # memset clears garbage stripes (read by the oc0 K=128 fold).
            tT4 = tpool.tile([P, M], BF16, tag="tT")
            nc.vector.memset(tT4[:], 0.0)
            # tTp [128, M]: stripes packed to rows {16j..16j+15} and
            # duplicated at rows 64+, for the row-tiled K=64 pair fold.
            tTp = tpool.tile([P, M], BF16, tag="tP")

            # HAM warmup: dummy matmuls so the PE clock ramps to 8/8.
            junk = ppool.tile([P, OC], F32, tag="acc", name="junk")
            for i in range(N_WARMUP):
                nc.tensor.matmul(
                    junk[:],
                    btile[:, :P],
                    btile[:, :OC],
                    start=(i == 0),
                    stop=(i == N_WARMUP - 1),
                )

            def w_load(oc):
                wtiles = []
                for k4 in range(N_KQ):
                    wtile = wpool.tile([P, KQ * OC], BF16, tag="wt")
                    nc.sync.dma_start(
                        wtile.rearrange("p (b c) -> p b c", b=KQ),
                        wt[k4 * KQ * P:(k4 + 1) * KQ * P,
                           oc * OC:(oc + 1) * OC].rearrange(
                               "(b p) c -> p b c", p=P),
                    )
                    wtiles.append(wtile)
                return wtiles

            def finish_quarter(oc, q, psq, packed):
                ot = opool.tile([P, MQ * OC], F32, tag="ot")
                if packed:
                    # row-tiled K=64 pairs: even mi rows 0-63, odd 64-127
                    for mi in range(MQ):
                        mt = q * MQ + mi
                        r0 = 0 if mi % 2 == 0 else 64
                        nc.tensor.matmul(
                            psq[mi][:],
                            tTp[r0:r0 + 64, mt * P:(mt + 1) * P],
                            atile_p[r0:r0 + 64, oc * OC:(oc + 1) * OC],
                            start=False,
                            stop=True,
                            tile_position=(r0, 0),
                        )
                else:
                    for mi in range(MQ):
                        mt = q * MQ + mi
                        nc.tensor.matmul(
                            psq[mi][:],
                            tT4[:, mt * P:(mt + 1) * P],
                            atile_r[:, oc * OC:(oc + 1) * OC],
                            start=False,
                            stop=True,
                        )
                for mi in range(MQ):
                    nc.vector.tensor_copy(
                        ot[:, mi * OC:(mi + 1) * OC], psq[mi][:]
                    )
                seng = nc.scalar if (oc * NQ + q) % 2 == 0 else nc.sync
                seng.dma_start(
                    out[q * MQ * P:(q + 1) * MQ * P,
                        oc * OC:(oc + 1) * OC].rearrange(
                            "(b p) c -> p b c", p=P),
                    ot.rearrange("p (b c) -> p b c", b=MQ),
                )

            # oc0 fused with stage A: per k, 1 stage-A MM + 4 main MMs,
            # all gated on the same arriving x tile (k, mq=g).
            wtiles = w_load(0)
            for g in range(NQ):
                pa = [
                    ppool.tile([P, OC], F32, tag="acc", name=f"pa_{g}_{j}")
                    for j in range(4)
                ]
                psq = [
                    ppool.tile([P, OC], F32, tag="acc", name=f"ps_0_{g}_{mi}")
                    for mi in range(MQ)
                ]
                # k-quad interleave: 4 adjacent col-tiled stage-A MMs
                # (keeps their 4-way overlap), then 16 main MMs — all
                # gated on the same 4 arriving x tiles.
                for k4 in range(N_KQ):
                    for kk in range(KQ):
                        k = KQ * k4 + kk
                        j = k % 4
                        nc.tensor.matmul(
                            pa[j][32 * j:32 * j + RANK, :],
                            btile[:, k * RANK:(k + 1) * RANK],
                            xsl(k, g * OC, OC),
                            start=(k < 4),
                            stop=(k >= KT - 4),
                            tile_position=(0, 32 * j),
                        )
                    for kk in range(KQ):
                        k = KQ * k4 + kk
                        for mi in range(MQ):
                            mt = g * MQ + mi
                            nc.tensor.matmul(
                                psq[mi][:],
                                xsl(k, mt * P, P),
                                wtiles[k4][:, kk * OC:(kk + 1) * OC],
                                start=(k == 0),
                                stop=False,
                            )
                for j in range(4):
                    nc.vector.tensor_copy(
                        tT4[32 * j:32 * j + RANK, g * OC:(g + 1) * OC],
                        pa[j][32 * j:32 * j + RANK, :],
                    )
                # pack + duplicate stripes into tTp (SBUF->SBUF, sync queue)
                for j in range(4):
                    sstr = tT4[32 * j:32 * j + RANK, g * OC:(g + 1) * OC]
                    nc.gpsimd.dma_start(
                        tTp[16 * j:16 * j + RANK, g * OC:(g + 1) * OC], sstr)
                    nc.gpsimd.dma_start(
                        tTp[64 + 16 * j:64 + 16 * j + RANK,
                            g * OC:(g + 1) * OC], sstr)
                finish_quarter(0, g, psq, packed=False)

            for oc in range(1, N_OC):
                wtiles = w_load(oc)
                for q in range(NQ):
                    psq = [
                        ppool.tile([P, OC], F32, tag="acc",
                                   name=f"ps_{oc}_{q}_{mi}")
                        for mi in range(MQ)
                    ]
                    for k4 in range(N_KQ):
                        for kk in range(KQ):
                            k = KQ * k4 + kk
                            for mi in range(MQ):
                                mt = q * MQ + mi
                                nc.tensor.matmul(
                                    psq[mi][:],
                                    xsl(k, mt * P, P),
                                    wtiles[k4][:, kk * OC:(kk + 1) * OC],
                                    start=(k == 0),
                                    stop=False,
                                )
                    finish_quarter(oc, q, psq, packed=True)

    split_wide_waits(nc)
    return nc


_NC_CACHE = [None]


def kernel(x, weight, lora_A, lora_B):
    from concourse.bass_utils import run_bass_kernel_spmd

    x = np.asarray(x, dtype=np.float32)
    weight = np.asarray(weight, dtype=np.float32)
    lora_A = np.asarray(lora_A, dtype=np.float32)
    lora_B = np.asarray(lora_B, dtype=np.float32)

    x2 = x.reshape(ROWS_TOTAL, D)
    wt = np.ascontiguousarray(weight.T).astype(BF16_NP)
    a2 = (2.0 * lora_A).astype(BF16_NP)
    # a2rep: 2*A at stripes {32j..32j+15}, zeros elsewhere
    a2rep = np.zeros((P, D), dtype=BF16_NP)
    for j in range(4):
        a2rep[32 * j:32 * j + RANK, :] = a2
    # a2p: 2*A tiled at rows {16j..16j+15} and duplicated at rows 64+
    a2p = np.zeros((P, D), dtype=BF16_NP)
    for j in range(4):
        a2p[16 * j:16 * j + RANK, :] = a2
        a2p[64 + 16 * j:64 + 16 * j + RANK, :] = a2
    # pre-arrange B: [128, KT*RANK], col-block k holds rows k*128..(k+1)*128
    bmat = np.ascontiguousarray(
        lora_B.reshape(KT, P, RANK).transpose(1, 0, 2).reshape(P, KT * RANK)
    ).astype(BF16_NP)

    in_maps = []
    for c in range(N_CORES):
        xt_c = np.ascontiguousarray(
            x2[c * M:(c + 1) * M].T
        ).astype(BF16_NP)
        in_maps.append({"xt": xt_c, "wt": wt, "bmat": bmat,
                        "a2rep": a2rep, "a2p": a2p})

    if _NC_CACHE[0] is None:
        _NC_CACHE[0] = build_program()
    nc = _NC_CACHE[0]

    res = run_bass_kernel_spmd(nc, in_maps, list(range(N_CORES)))
    out = np.concatenate(
        [res.results[c]["out"] for c in range(N_CORES)], axis=0
    )
    return out.reshape(x.shape)



# revision 3
# speedup vs baseline: 1.2507x; 1.2507x over previous
"""LoRA layer kernel for Trainium2, 8-core data-parallel.

out = x @ W.T + 2.0 * ((x @ B) @ A)
  x: (4, 4096, 4096) f32, W: (4096, 4096), A: (16, 4096), B: (4096, 16)

Strategy: fold the LoRA term into the weight on the host
(WF = W.T + 2*B@A, exact same math, 0.5 GFLOP of numpy), so the device
kernel is a single dense GEMM out = x @ WF. Flatten x to (16384, 4096)
rows, shard rows across 8 cores (2048 rows each), replicate WF. All
matmul operands bf16 (PSUM accumulation fp32; rel err ~2e-3 vs the
2e-2 gate).

Per core, single x-resident block (2048 rows = 128 KB/partition bf16):
  - warmup: 8 junk matmuls on a memset tile (no DMA dependency) so the
    PE HAM clock ramps to 8/8 while the first x/W blocks are in flight.
  - x loads m-split into 512-col quarters, laid out [k4][mq][kk][m] so
    each (k-quad, m-quarter) block is one flat 512KB DMA; the 8 blocks
    of quarter 0 are interleaved with the 8 W(oc0) quads so the PE's
    per-k4 needs (x block + W quad, 1MB / ~4.2us) arrive in order well
    under the ~320 GB/s DMA rate.
  - main GEMM: per o-chunk (512 cols, one PSUM bank), 4 m-quarters of
    4 PSUM banks each (4+4 double buffering: quarter q+1's banks were
    drained during quarter q-1, so bank reuse never stalls the PE);
    x-tile stationary, WF streamed as k-quads [128, 4*512].
  - last quarter (oc7,q3) runs mi-outer/k-inner (all data resident) so
    each PSUM bank finishes 32 MMs before the next starts: drain+store
    pipelines per-mi and the tail is one copy + one 256KB DMA (~1.5us)
    instead of 4 copies + a 1MB DMA (~8us).
Queues: W on sync, x loads round-robin gpsimd/scalar/vector, out
stores alternate scalar/sync.
"""

import sys

if "/opt/trn_rl_repo" not in sys.path:
    sys.path.insert(0, "/opt/trn_rl_repo")

import numpy as np
import ml_dtypes

import concourse.bass as bass
import concourse.mybir as mybir
import concourse.tile as tile

N_CORES = 8
D = 4096
RANK = 16
ROWS_TOTAL = 4 * 4096          # 16384
M = ROWS_TOTAL // N_CORES      # 2048 rows per core
P = 128
KT = D // P                    # 32 k-tiles
OC = 512                       # o-chunk width (one PSUM bank)
N_OC = D // OC                 # 8
MT = M // P                    # 16 m-tiles
MQ = 4                         # m-tiles per quarter (PSUM banks)
NQ = MT // MQ                  # 4 quarters
KQ = 4                         # k-tiles per W quad DMA
N_KQ = KT // KQ                # 8

F32 = mybir.dt.float32
BF16 = mybir.dt.bfloat16
BF16_NP = ml_dtypes.bfloat16

N_WARMUP = 8


def split_wide_waits(nc, max_waits=1):
    """walrus in this container rejects >1 sync wait per instruction;
    move excess waits onto preceding same-engine NoOps."""
    n_split = 0
    for f in nc.m.functions:
        for bb in f.blocks:
            new_insts = []
            for inst in bb.instructions:
                si = getattr(inst, "sync_info", None)
                if si is not None and si.on_wait and len(si.on_wait) > max_waits:
                    waits = list(si.on_wait)
                    keep = waits[-max_waits:]
                    extra = waits[:-max_waits]
                    for i in range(0, len(extra), max_waits):
                        chunk = extra[i:i + max_waits]
                        nop = mybir.InstNoOp(
                            name=f"{inst.name}_wsplit{i}",
                            sync_info=mybir.SyncInfo(on_wait=chunk, on_update=[]),
                            bass_nofuse=True,
                            engine=inst.engine,
                        )
                        new_insts.append(nop)
                        n_split += 1
                    si.on_wait = keep
                new_insts.append(inst)
            bb.instructions[:] = new_insts
    return n_split


def build_program():
    nc = bass.Bass()
    xt = nc.declare_dram_parameter("xt", [D, M], BF16, isOutput=False)
    wf = nc.declare_dram_parameter("wf", [D, D], BF16, isOutput=False)
    out = nc.declare_dram_parameter("out", [M, D], F32, isOutput=True)

    with tile.TileContext(nc) as tc:
        with (
            tc.tile_pool(name="xpool", bufs=1) as xpool,
            tc.tile_pool(name="wpool", bufs=9) as wpool,
            tc.tile_pool(name="opool", bufs=3) as opool,
            tc.tile_pool(name="cpool", bufs=1) as cpool,
            tc.tile_pool(name="ppool", bufs=8, space="PSUM") as ppool,
        ):
            # HAM warmup on a memset tile: no DMA dependency, so the PE
            # is busy (and ramping to 8/8) from ~t0 while x/W stream in.
            jt = cpool.tile([P, OC], BF16, tag="junk_src")
            nc.vector.memset(jt[:], 0.0)
            junk = ppool.tile([P, OC], F32, tag="acc", name="junk")
            for i in range(N_WARMUP):
                nc.tensor.matmul(
                    junk[:],
                    jt[:, :P],
                    jt[:],
                    start=(i == 0),
                    stop=(i == N_WARMUP - 1),
                )

            # x fully resident, laid out [k4][mq][kk][m] so each
            # k-quad x m-quarter block is flat-contiguous: one 512KB DMA
            # per block (32 total), arriving at exactly the granularity
            # the PE consumes (16 main MMs per block).
            xall = xpool.tile([P, KT * M], BF16, tag="x")

            def xsl(k, c0, cw):
                k4, kk = divmod(k, KQ)
                mq, d = divmod(c0, OC)
                assert d + cw <= OC
                base = ((k4 * NQ + mq) * KQ + kk) * OC + d
                return xall[:, base: base + cw]

            def x_load(mq, k4, eng):
                base = (k4 * NQ + mq) * KQ * OC
                eng.dma_start(
                    xall[:, base: base + KQ * OC].rearrange(
                        "p (b c) -> p b c", b=KQ),
                    xt[k4 * KQ * P:(k4 + 1) * KQ * P,
                       mq * OC:(mq + 1) * OC].rearrange(
                           "(b p) c -> p b c", p=P),
                )

            def w_quad(oc, k4):
                wtile = wpool.tile([P, KQ * OC], BF16, tag="wt")
                nc.sync.dma_start(
                    wtile.rearrange("p (b c) -> p b c", b=KQ),
                    wf[k4 * KQ * P:(k4 + 1) * KQ * P,
                       oc * OC:(oc + 1) * OC].rearrange(
                           "(b p) c -> p b c", p=P),
                )
                return wtile

            # startup: interleave x quarter-0 blocks with W(oc0) quads so
            # the PE's k4-order needs land pairwise; remaining x quarters
            # stream behind on 3 queues.
            xengs = [nc.gpsimd, nc.scalar]
            wtiles0 = []
            for k4 in range(N_KQ):
                x_load(0, k4, xengs[k4 % 2])
                wtiles0.append(w_quad(0, k4))
            nx = 0
            for mq in range(1, NQ):
                for k4 in range(N_KQ):
                    x_load(mq, k4, xengs[nx % 2])
                    nx += 1

            def quarter_mms(wtiles, q, psq):
                for k4 in range(N_KQ):
                    for kk in range(KQ):
                        k = KQ * k4 + kk
                        for mi in range(MQ):
                            mt = q * MQ + mi
                            nc.tensor.matmul(
                                psq[mi][:],
                                xsl(k, mt * P, P),
                                wtiles[k4][:, kk * OC:(kk + 1) * OC],
                                start=(k == 0),
                                stop=(k == KT - 1),
                            )

            def out_dma(eng, oc, q, ot, mi=None):
                if mi is None:
                    eng.dma_start(
                        out[q * MQ * P:(q + 1) * MQ * P,
                            oc * OC:(oc + 1) * OC].rearrange(
                                "(b p) c -> p b c", p=P),
                        ot.rearrange("p (b c) -> p b c", b=MQ),
                    )
                else:
                    mt = q * MQ + mi
                    eng.dma_start(
                        out[mt * P:(mt + 1) * P, oc * OC:(oc + 1) * OC],
                        ot[:, mi * OC:(mi + 1) * OC],
                    )

            for oc in range(N_OC):
                wtiles = wtiles0 if oc == 0 else [
                    w_quad(oc, k4) for k4 in range(N_KQ)
                ]
                for q in range(NQ):
                    psq = [
                        ppool.tile([P, OC], F32, tag="acc",
                                   name=f"ps_{oc}_{q}_{mi}")
                        for mi in range(MQ)
                    ]
                    last = (oc == N_OC - 1 and q == NQ - 1)
                    ot = opool.tile([P, MQ * OC], F32, tag="ot")
                    if not last:
                        quarter_mms(wtiles, q, psq)
                        for mi in range(MQ):
                            nc.vector.tensor_copy(
                                ot[:, mi * OC:(mi + 1) * OC], psq[mi][:]
                            )
                        seng = nc.scalar if (oc * NQ + q) % 2 == 0 else nc.sync
                        out_dma(seng, oc, q, ot)
                    else:
                        # mi-outer so each PSUM bank completes its 32-MM
                        # accumulation early; drain+store pipeline per-mi
                        # cuts the kernel tail to one copy + one 256KB DMA.
                        for mi in range(MQ):
                            mt = q * MQ + mi
                            for k4 in range(N_KQ):
                                for kk in range(KQ):
                                    k = KQ * k4 + kk
                                    nc.tensor.matmul(
                                        psq[mi][:],
                                        xsl(k, mt * P, P),
                                        wtiles[k4][:, kk * OC:(kk + 1) * OC],
                                        start=(k == 0),
                                        stop=(k == KT - 1),
                                    )
                            nc.vector.tensor_copy(
                                ot[:, mi * OC:(mi + 1) * OC], psq[mi][:]
                            )
                            seng = nc.scalar if mi % 2 == 0 else nc.sync
                            out_dma(seng, oc, q, ot, mi=mi)

    split_wide_waits(nc)
    return nc


_NC_CACHE = [None]


def kernel(x, weight, lora_A, lora_B):
    from concourse.bass_utils import run_bass_kernel_spmd

    x = np.asarray(x, dtype=np.float32)
    weight = np.asarray(weight, dtype=np.float32)
    lora_A = np.asarray(lora_A, dtype=np.float32)
    lora_B = np.asarray(lora_B, dtype=np.float32)

    x2 = x.reshape(ROWS_TOTAL, D)
    # fold LoRA into the weight: out = x @ (W.T + 2*B@A), exact rewrite
    wf = np.ascontiguousarray(
        weight.T + 2.0 * (lora_B @ lora_A)
    ).astype(BF16_NP)

    in_maps = []
    for c in range(N_CORES):
        xt_c = np.ascontiguousarray(
            x2[c * M:(c + 1) * M].T
        ).astype(BF16_NP)
        in_maps.append({"xt": xt_c, "wf": wf})

    if _NC_CACHE[0] is None:
        _NC_CACHE[0] = build_program()
    nc = _NC_CACHE[0]

    res = run_bass_kernel_spmd(nc, in_maps, list(range(N_CORES)))
    out = np.concatenate(
        [res.results[c]["out"] for c in range(N_CORES)], axis=0
    )
    return out.reshape(x.shape)


# revision 6
# speedup vs baseline: 1.2567x; 1.0048x over previous
"""LoRA layer kernel for Trainium2, 8-core data-parallel.

out = x @ W.T + 2.0 * ((x @ B) @ A)
  x: (4, 4096, 4096) f32, W: (4096, 4096), A: (16, 4096), B: (4096, 16)

Strategy: fold the LoRA term into the weight on the host
(WF = W.T + 2*B@A, exact same math, 0.5 GFLOP of numpy), so the device
kernel is a single dense GEMM out = x @ WF. Flatten x to (16384, 4096)
rows, shard rows across 8 cores (2048 rows each), replicate WF. All
matmul operands bf16 (PSUM accumulation fp32; rel err ~2e-3 vs the
2e-2 gate).

Per core, single x-resident block (2048 rows = 128 KB/partition bf16):
  - warmup: 8 junk matmuls on a memset tile (no DMA dependency) so the
    PE HAM clock ramps to 8/8 while the first x/W blocks are in flight.
  - x loads m-split into 512-col quarters, laid out [k4][mq][kk][m] so
    each (k-quad, m-quarter) block is one flat 512KB DMA; the 8 blocks
    of quarter 0 are interleaved with the 8 W(oc0) quads so the PE's
    per-k4 needs (x block + W quad, 1MB / ~4.2us) arrive in order well
    under the ~320 GB/s DMA rate.
  - main GEMM: per o-chunk (512 cols, one PSUM bank), 4 m-quarters of
    4 PSUM banks each (4+4 double buffering: quarter q+1's banks were
    drained during quarter q-1, so bank reuse never stalls the PE);
    x-tile stationary, WF streamed as k-quads [128, 4*512].
  - last quarter (oc7,q3) runs mi-outer/k-inner (all data resident) so
    each PSUM bank finishes 32 MMs before the next starts: drain+store
    pipelines per-mi and the tail is one copy + one 256KB DMA (~1.5us)
    instead of 4 copies + a 1MB DMA (~8us).
Queues: W on sync, x loads round-robin gpsimd/scalar/vector, out
stores alternate scalar/sync.
"""

import sys

if "/opt/trn_rl_repo" not in sys.path:
    sys.path.insert(0, "/opt/trn_rl_repo")

import numpy as np
import ml_dtypes

import concourse.bass as bass
import concourse.mybir as mybir
import concourse.tile as tile

N_CORES = 8
D = 4096
RANK = 16
ROWS_TOTAL = 4 * 4096          # 16384
M = ROWS_TOTAL // N_CORES      # 2048 rows per core
P = 128
KT = D // P                    # 32 k-tiles
OC = 512                       # o-chunk width (one PSUM bank)
N_OC = D // OC                 # 8
MT = M // P                    # 16 m-tiles
MQ = 4                         # m-tiles per quarter (PSUM banks)
NQ = MT // MQ                  # 4 quarters
KQ = 4                         # k-tiles per W quad DMA
N_KQ = KT // KQ                # 8

F32 = mybir.dt.float32
BF16 = mybir.dt.bfloat16
BF16_NP = ml_dtypes.bfloat16

N_WARMUP = 8


def split_wide_waits(nc, max_waits=1):
    """walrus in this container rejects >1 sync wait per instruction;
    move excess waits onto preceding same-engine NoOps."""
    n_split = 0
    for f in nc.m.functions:
        for bb in f.blocks:
            new_insts = []
            for inst in bb.instructions:
                si = getattr(inst, "sync_info", None)
                if si is not None and si.on_wait and len(si.on_wait) > max_waits:
                    waits = list(si.on_wait)
                    keep = waits[-max_waits:]
                    extra = waits[:-max_waits]
                    for i in range(0, len(extra), max_waits):
                        chunk = extra[i:i + max_waits]
                        nop = mybir.InstNoOp(
                            name=f"{inst.name}_wsplit{i}",
                            sync_info=mybir.SyncInfo(on_wait=chunk, on_update=[]),
                            bass_nofuse=True,
                            engine=inst.engine,
                        )
                        new_insts.append(nop)
                        n_split += 1
                    si.on_wait = keep
                new_insts.append(inst)
            bb.instructions[:] = new_insts
    return n_split


def build_program():
    nc = bass.Bass()
    xt = nc.declare_dram_parameter("xt", [D, M], BF16, isOutput=False)
    wf = nc.declare_dram_parameter("wf", [D, D], BF16, isOutput=False)
    out = nc.declare_dram_parameter("out", [M, D], F32, isOutput=True)

    with tile.TileContext(nc) as tc:
        with (
            tc.tile_pool(name="xpool", bufs=1) as xpool,
            tc.tile_pool(name="wpool", bufs=9) as wpool,
            tc.tile_pool(name="opool", bufs=3) as opool,
            tc.tile_pool(name="cpool", bufs=1) as cpool,
            tc.tile_pool(name="ppool", bufs=8, space="PSUM") as ppool,
        ):
            # HAM warmup on a memset tile: no DMA dependency, so the PE
            # is busy (and ramping to 8/8) from ~t0 while x/W stream in.
            jt = cpool.tile([P, OC], BF16, tag="junk_src")
            nc.vector.memset(jt[:], 0.0)
            junk = ppool.tile([P, OC], F32, tag="acc", name="junk")
            for i in range(N_WARMUP):
                nc.tensor.matmul(
                    junk[:],
                    jt[:, :P],
                    jt[:],
                    start=(i == 0),
                    stop=(i == N_WARMUP - 1),
                )

            # x fully resident, laid out [k4][mq][kk][m] so each
            # k-quad x m-quarter block is flat-contiguous: one 512KB DMA
            # per block (32 total), arriving at exactly the granularity
            # the PE consumes (16 main MMs per block).
            xall = xpool.tile([P, KT * M], BF16, tag="x")

            def xsl(k, c0, cw):
                k4, kk = divmod(k, KQ)
                mq, d = divmod(c0, OC)
                assert d + cw <= OC
                base = ((k4 * NQ + mq) * KQ + kk) * OC + d
                return xall[:, base: base + cw]

            def x_load(mq, k4, eng):
                base = (k4 * NQ + mq) * KQ * OC
                eng.dma_start(
                    xall[:, base: base + KQ * OC].rearrange(
                        "p (b c) -> p b c", b=KQ),
                    xt[k4 * KQ * P:(k4 + 1) * KQ * P,
                       mq * OC:(mq + 1) * OC].rearrange(
                           "(b p) c -> p b c", p=P),
                )

            def w_quad(oc, k4, eng=None):
                wtile = wpool.tile([P, KQ * OC], BF16, tag="wt")
                (eng or nc.sync).dma_start(
                    wtile.rearrange("p (b c) -> p b c", b=KQ),
                    wf[k4 * KQ * P:(k4 + 1) * KQ * P,
                       oc * OC:(oc + 1) * OC].rearrange(
                           "(b p) c -> p b c", p=P),
                )
                return wtile

            # startup: enqueue blocks in exact PE need-order, striped
            # round-robin over the 3 DMA rings (sync/scalar HWDGE +
            # gpsimd SWDGE) so packet-level ring round-robin delivers
            # them in need-order at aggregate HBM bandwidth: pairs
            # (x(mq0,k4), W(oc0,k4)) first, then x quarters 1-3.
            dengs = [nc.sync, nc.scalar, nc.gpsimd]
            wtiles0 = [None] * N_KQ
            nj = 0
            for k4 in range(N_KQ):
                x_load(0, k4, dengs[nj % 3]); nj += 1
                wtiles0[k4] = w_quad(0, k4, dengs[nj % 3]); nj += 1
            for mq in range(1, NQ):
                for k4 in range(N_KQ):
                    x_load(mq, k4, dengs[nj % 3]); nj += 1

            def quarter_mms(wtiles, q, psq):
                for k4 in range(N_KQ):
                    for kk in range(KQ):
                        k = KQ * k4 + kk
                        for mi in range(MQ):
                            mt = q * MQ + mi
                            nc.tensor.matmul(
                                psq[mi][:],
                                xsl(k, mt * P, P),
                                wtiles[k4][:, kk * OC:(kk + 1) * OC],
                                start=(k == 0),
                                stop=(k == KT - 1),
                            )

            def out_dma(eng, oc, q, ot, mi=None):
                if mi is None:
                    eng.dma_start(
                        out[q * MQ * P:(q + 1) * MQ * P,
                            oc * OC:(oc + 1) * OC].rearrange(
                                "(b p) c -> p b c", p=P),
                        ot.rearrange("p (b c) -> p b c", b=MQ),
                    )
                else:
                    mt = q * MQ + mi
                    eng.dma_start(
                        out[mt * P:(mt + 1) * P, oc * OC:(oc + 1) * OC],
                        ot[:, mi * OC:(mi + 1) * OC],
                    )

            for oc in range(N_OC):
                wtiles = wtiles0 if oc == 0 else [
                    w_quad(oc, k4) for k4 in range(N_KQ)
                ]
                for q in range(NQ):
                    psq = [
                        ppool.tile([P, OC], F32, tag="acc",
                                   name=f"ps_{oc}_{q}_{mi}")
                        for mi in range(MQ)
                    ]
                    last = (oc == N_OC - 1 and q == NQ - 1)
                    ot = opool.tile([P, MQ * OC], F32, tag="ot")
                    if not last:
                        quarter_mms(wtiles, q, psq)
                        for mi in range(MQ):
                            nc.vector.tensor_copy(
                                ot[:, mi * OC:(mi + 1) * OC], psq[mi][:]
                            )
                        seng = nc.scalar if (oc * NQ + q) % 2 == 0 else nc.sync
                        out_dma(seng, oc, q, ot)
                    else:
                        # mi-outer so each PSUM bank completes its 32-MM
                        # accumulation early; drain+store pipeline per-mi
                        # cuts the kernel tail to one copy + one 256KB DMA.
                        for mi in range(MQ):
                            mt = q * MQ + mi
                            for k4 in range(N_KQ):
                                for kk in range(KQ):
                                    k = KQ * k4 + kk
                                    nc.tensor.matmul(
                                        psq[mi][:],
                                        xsl(k, mt * P, P),
                                        wtiles[k4][:, kk * OC:(kk + 1) * OC],
                                        start=(k == 0),
                                        stop=(k == KT - 1),
                                    )
                            if mi < MQ - 1:
                                nc.vector.tensor_copy(
                                    ot[:, mi * OC:(mi + 1) * OC], psq[mi][:]
                                )
                                seng = nc.scalar if mi % 2 == 0 else nc.sync
                                out_dma(seng, oc, q, ot, mi=mi)
                            else:
                                # split the very last drain in half so the
                                # kernel tail is one 128x256 copy + 128KB DMA
                                mt = q * MQ + mi
                                H = OC // 2
                                for h in range(2):
                                    nc.vector.tensor_copy(
                                        ot[:, mi * OC + h * H:
                                           mi * OC + (h + 1) * H],
                                        psq[mi][:, h * H:(h + 1) * H],
                                    )
                                    (nc.sync if h == 0 else nc.scalar).dma_start(
                                        out[mt * P:(mt + 1) * P,
                                            oc * OC + h * H:
                                            oc * OC + (h + 1) * H],
                                        ot[:, mi * OC + h * H:
                                           mi * OC + (h + 1) * H],
                                    )

    split_wide_waits(nc)
    return nc


_NC_CACHE = [None]


def kernel(x, weight, lora_A, lora_B):
    from concourse.bass_utils import run_bass_kernel_spmd

    x = np.asarray(x, dtype=np.float32)
    weight = np.asarray(weight, dtype=np.float32)
    lora_A = np.asarray(lora_A, dtype=np.float32)
    lora_B = np.asarray(lora_B, dtype=np.float32)

    x2 = x.reshape(ROWS_TOTAL, D)
    # fold LoRA into the weight: out = x @ (W.T + 2*B@A), exact rewrite
    wf = np.ascontiguousarray(
        weight.T + 2.0 * (lora_B @ lora_A)
    ).astype(BF16_NP)

    in_maps = []
    for c in range(N_CORES):
        xt_c = np.ascontiguousarray(
            x2[c * M:(c + 1) * M].T
        ).astype(BF16_NP)
        in_maps.append({"xt": xt_c, "wf": wf})

    if _NC_CACHE[0] is None:
        _NC_CACHE[0] = build_program()
    nc = _NC_CACHE[0]

    res = run_bass_kernel_spmd(nc, in_maps, list(range(N_CORES)))
    out = np.concatenate(
        [res.results[c]["out"] for c in range(N_CORES)], axis=0
    )
    return out.reshape(x.shape)


# revision 15
# speedup vs baseline: 1.2650x; 1.0066x over previous
"""LoRA layer kernel for Trainium2, 8-core data-parallel.

out = x @ W.T + 2.0 * ((x @ B) @ A)
  x: (4, 4096, 4096) f32, W: (4096, 4096), A: (16, 4096), B: (4096, 16)

Strategy: fold the LoRA term into the weight on the host
(WF = W.T + 2*B@A, exact same math, 0.5 GFLOP of numpy), so the device
kernel is a single dense GEMM out = x @ WF. Flatten x to (16384, 4096)
rows, shard rows across 8 cores (2048 rows each), replicate WF. All
matmul operands bf16 (PSUM accumulation fp32; rel err ~2e-3 vs the
2e-2 gate).

Per core, single x-resident block (2048 rows = 128 KB/partition bf16):
  - warmup: 8 junk matmuls on a memset tile (no DMA dependency) so the
    PE HAM clock ramps to 8/8 while the first x/W blocks are in flight.
  - x loads m-split into 512-col quarters, laid out [k4][mq][kk][m] so
    each (k-quad, m-quarter) block is one flat 512KB DMA.
  - DMA ring capabilities (measured): gpsimd SWDGE ~210 GB/s, each
    HWDGE ring (sync, scalar) only ~58 GB/s. Quarter 0 consumes x and
    W(oc0) at ~148 GB/s each, so: all x on gpsimd in PE need-order
    with W(oc0) quads 0/3/6 interleaved at their need points; W(oc0)
    quads 1/4/7 on sync, 2/5 on scalar; W(oc>=1) alternates sync/
    scalar (wpool buffer frees pace them k4-aligned); out rides
    gpsimd, which is idle once x drains (~85us).
  - main GEMM: per o-chunk (512 cols, one PSUM bank), 4 m-quarters of
    4 PSUM banks each (4+4 double buffering: quarter q+1's banks were
    drained during quarter q-1, so bank reuse never stalls the PE);
    x-tile stationary, WF streamed as k-quads [128, 4*512].
  - last quarter (oc7,q3) runs mi-outer/k-inner (all data resident) so
    each PSUM bank finishes 32 MMs before the next starts: drain+store
    pipelines per-mi and the tail is two 128x256 copy+DMA halves
    (~1.5us) instead of 4 copies + a 1MB DMA (~8us).
"""

import sys

if "/opt/trn_rl_repo" not in sys.path:
    sys.path.insert(0, "/opt/trn_rl_repo")

import numpy as np
import ml_dtypes

import concourse.bass as bass
import concourse.mybir as mybir
import concourse.tile as tile

N_CORES = 8
D = 4096
RANK = 16
ROWS_TOTAL = 4 * 4096          # 16384
M = ROWS_TOTAL // N_CORES      # 2048 rows per core
P = 128
KT = D // P                    # 32 k-tiles
OC = 512                       # o-chunk width (one PSUM bank)
N_OC = D // OC                 # 8
MT = M // P                    # 16 m-tiles
MQ = 4                         # m-tiles per quarter (PSUM banks)
NQ = MT // MQ                  # 4 quarters
KQ = 4                         # k-tiles per W quad DMA
N_KQ = KT // KQ                # 8

F32 = mybir.dt.float32
BF16 = mybir.dt.bfloat16
BF16_NP = ml_dtypes.bfloat16

N_WARMUP = 8


def split_wide_waits(nc, max_waits=1):
    """walrus in this container rejects >1 sync wait per instruction;
    move excess waits onto preceding same-engine NoOps."""
    n_split = 0
    for f in nc.m.functions:
        for bb in f.blocks:
            new_insts = []
            for inst in bb.instructions:
                si = getattr(inst, "sync_info", None)
                if si is not None and si.on_wait and len(si.on_wait) > max_waits:
                    waits = list(si.on_wait)
                    keep = waits[-max_waits:]
                    extra = waits[:-max_waits]
                    for i in range(0, len(extra), max_waits):
                        chunk = extra[i:i + max_waits]
                        nop = mybir.InstNoOp(
                            name=f"{inst.name}_wsplit{i}",
                            sync_info=mybir.SyncInfo(on_wait=chunk, on_update=[]),
                            bass_nofuse=True,
                            engine=inst.engine,
                        )
                        new_insts.append(nop)
                        n_split += 1
                    si.on_wait = keep
                new_insts.append(inst)
            bb.instructions[:] = new_insts
    return n_split


def build_program():
    nc = bass.Bass()
    xt = nc.declare_dram_parameter("xt", [D, M], BF16, isOutput=False)
    wf = nc.declare_dram_parameter("wf", [D, D], BF16, isOutput=False)
    out = nc.declare_dram_parameter("out", [M, D], F32, isOutput=True)

    with tile.TileContext(nc) as tc:
        with (
            tc.tile_pool(name="xpool", bufs=1) as xpool,
            tc.tile_pool(name="wpool", bufs=9) as wpool,
            tc.tile_pool(name="opool", bufs=4) as opool,
            tc.tile_pool(name="cpool", bufs=1) as cpool,
            tc.tile_pool(name="ppool", bufs=8, space="PSUM") as ppool,
        ):
            # HAM warmup on a memset tile: no DMA dependency, so the PE
            # is busy (and ramping to 8/8) from ~t0 while x/W stream in.
            jt = cpool.tile([P, OC], BF16, tag="junk_src")
            nc.vector.memset(jt[:], 0.0)
            junk = ppool.tile([P, OC], F32, tag="acc", name="junk")
            for i in range(N_WARMUP):
                nc.tensor.matmul(
                    junk[:],
                    jt[:, :P],
                    jt[:],
                    start=(i == 0),
                    stop=(i == N_WARMUP - 1),
                )

            # x fully resident, laid out [k4][mq][kk][m] so each
            # k-quad x m-quarter block is flat-contiguous: one 512KB DMA
            # per block (32 total), arriving at exactly the granularity
            # the PE consumes (16 main MMs per block).
            xall = xpool.tile([P, KT * M], BF16, tag="x")

            def xsl(k, c0, cw):
                k4, kk = divmod(k, KQ)
                mq, d = divmod(c0, OC)
                assert d + cw <= OC
                base = ((k4 * NQ + mq) * KQ + kk) * OC + d
                return xall[:, base: base + cw]

            def x_load(mq, k4, eng):
                base = (k4 * NQ + mq) * KQ * OC
                eng.dma_start(
                    xall[:, base: base + KQ * OC].rearrange(
                        "p (b c) -> p b c", b=KQ),
                    xt[k4 * KQ * P:(k4 + 1) * KQ * P,
                       mq * OC:(mq + 1) * OC].rearrange(
                           "(b p) c -> p b c", p=P),
                )

            def w_dma(wtile, oc, k4, eng):
                eng.dma_start(
                    wtile.rearrange("p (b c) -> p b c", b=KQ),
                    wf[k4 * KQ * P:(k4 + 1) * KQ * P,
                       oc * OC:(oc + 1) * OC].rearrange(
                           "(b p) c -> p b c", p=P),
                )

            def w_quad(oc, k4, eng=None):
                wtile = wpool.tile([P, KQ * OC], BF16, tag="wt")
                w_dma(wtile, oc, k4, eng or nc.sync)
                return wtile

            # Ring capabilities (measured): gpsimd SWDGE ~210 GB/s;
            # each HWDGE ring (sync, scalar) only ~58 GB/s. Quarter 0
            # needs x at 148 GB/s + W(oc0) at 148 GB/s, so: all x on
            # gpsimd in need order, W(oc0) split 3 ways with quads
            # 0/3/6 interleaved into the gpsimd stream at their need
            # points; later W alternates sync/scalar; out rides gpsimd
            # (idle once x drains).
            # buffer allocation in k4 order (so W(oc1,k4) later waits on
            # the free of wtiles0[k4-1] — perfectly pipelined), DMAs
            # issued in per-ring need order.
            wtiles0 = [wpool.tile([P, KQ * OC], BF16, tag="wt",
                                  name=f"wt0_{k4}")
                       for k4 in range(N_KQ)]
            W0G = {0, 3, 6}
            for k4 in (1, 4, 7):
                w_dma(wtiles0[k4], 0, k4, nc.sync)
            for k4 in (2, 5):
                w_dma(wtiles0[k4], 0, k4, nc.scalar)
            for k4 in range(N_KQ):
                x_load(0, k4, nc.gpsimd)
                if k4 in W0G:
                    w_dma(wtiles0[k4], 0, k4, nc.gpsimd)
            for mq in range(1, NQ):
                for k4 in range(N_KQ):
                    x_load(mq, k4, nc.gpsimd)

            def quarter_mms(wtiles, q, psq):
                for k4 in range(N_KQ):
                    for kk in range(KQ):
                        k = KQ * k4 + kk
                        for mi in range(MQ):
                            mt = q * MQ + mi
                            nc.tensor.matmul(
                                psq[mi][:],
                                xsl(k, mt * P, P),
                                wtiles[k4][:, kk * OC:(kk + 1) * OC],
                                start=(k == 0),
                                stop=(k == KT - 1),
                            )

            def out_dma(eng, oc, q, ot, mi=None):
                if mi is None:
                    eng.dma_start(
                        out[q * MQ * P:(q + 1) * MQ * P,
                            oc * OC:(oc + 1) * OC].rearrange(
                                "(b p) c -> p b c", p=P),
                        ot.rearrange("p (b c) -> p b c", b=MQ),
                    )
                else:
                    mt = q * MQ + mi
                    eng.dma_start(
                        out[mt * P:(mt + 1) * P, oc * OC:(oc + 1) * OC],
                        ot[:, mi * OC:(mi + 1) * OC],
                    )

            for oc in range(N_OC):
                wtiles = wtiles0 if oc == 0 else [
                    w_quad(oc, k4, nc.sync if k4 % 2 == 0 else nc.scalar)
                    for k4 in range(N_KQ)
                ]
                for q in range(NQ):
                    psq = [
                        ppool.tile([P, OC], F32, tag="acc",
                                   name=f"ps_{oc}_{q}_{mi}")
                        for mi in range(MQ)
                    ]
                    last = (oc == N_OC - 1 and q == NQ - 1)
                    ot = opool.tile([P, MQ * OC], F32, tag="ot")
                    if not last:
                        quarter_mms(wtiles, q, psq)
                        for mi in range(MQ):
                            nc.vector.tensor_copy(
                                ot[:, mi * OC:(mi + 1) * OC], psq[mi][:]
                            )
                        # out rides the gpsimd ring: x occupies it only
                        # for the first ~85us and HWDGE stays free for W
                        out_dma(nc.gpsimd, oc, q, ot)
                    else:
                        # mi-outer so each PSUM bank completes its 32-MM
                        # accumulation early; drain+store pipeline per-mi
                        # cuts the kernel tail to one copy + one 256KB DMA.
                        for mi in range(MQ):
                            mt = q * MQ + mi
                            for k4 in range(N_KQ):
                                for kk in range(KQ):
                                    k = KQ * k4 + kk
                                    nc.tensor.matmul(
                                        psq[mi][:],
                                        xsl(k, mt * P, P),
                                        wtiles[k4][:, kk * OC:(kk + 1) * OC],
                                        start=(k == 0),
                                        stop=(k == KT - 1),
                                    )
                            if mi < MQ - 1:
                                nc.vector.tensor_copy(
                                    ot[:, mi * OC:(mi + 1) * OC], psq[mi][:]
                                )
                                seng = nc.gpsimd if mi % 2 == 0 else nc.scalar
                                out_dma(seng, oc, q, ot, mi=mi)
                            else:
                                # split the very last drain in half so the
                                # kernel tail is one 128x256 copy + 128KB DMA
                                mt = q * MQ + mi
                                H = OC // 2
                                for h in range(2):
                                    nc.vector.tensor_copy(
                                        ot[:, mi * OC + h * H:
                                           mi * OC + (h + 1) * H],
                                        psq[mi][:, h * H:(h + 1) * H],
                                    )
                                    (nc.sync if h == 0 else nc.scalar).dma_start(
                                        out[mt * P:(mt + 1) * P,
                                            oc * OC + h * H:
                                            oc * OC + (h + 1) * H],
                                        ot[:, mi * OC + h * H:
                                           mi * OC + (h + 1) * H],
                                    )

    split_wide_waits(nc)
    return nc


_NC_CACHE = [None]


def kernel(x, weight, lora_A, lora_B):
    from concourse.bass_utils import run_bass_kernel_spmd

    x = np.asarray(x, dtype=np.float32)
    weight = np.asarray(weight, dtype=np.float32)
    lora_A = np.asarray(lora_A, dtype=np.float32)
    lora_B = np.asarray(lora_B, dtype=np.float32)

    x2 = x.reshape(ROWS_TOTAL, D)
    # fold LoRA into the weight: out = x @ (W.T + 2*B@A), exact rewrite
    wf = np.ascontiguousarray(
        weight.T + 2.0 * (lora_B @ lora_A)
    ).astype(BF16_NP)

    in_maps = []
    for c in range(N_CORES):
        xt_c = np.ascontiguousarray(
            x2[c * M:(c + 1) * M].T
        ).astype(BF16_NP)
        in_maps.append({"xt": xt_c, "wf": wf})

    if _NC_CACHE[0] is None:
        _NC_CACHE[0] = build_program()
    nc = _NC_CACHE[0]

    res = run_bass_kernel_spmd(nc, in_maps, list(range(N_CORES)))
    out = np.concatenate(
        [res.results[c]["out"] for c in range(N_CORES)], axis=0
    )
    return out.reshape(x.shape)


# revision 19
# speedup vs baseline: 1.2674x; 1.0019x over previous
"""LoRA layer kernel for Trainium2, 8-core data-parallel.

out = x @ W.T + 2.0 * ((x @ B) @ A)
  x: (4, 4096, 4096) f32, W: (4096, 4096), A: (16, 4096), B: (4096, 16)

Strategy: fold the LoRA term into the weight on the host
(WF = W.T + 2*B@A, exact same math, 0.5 GFLOP of numpy), so the device
kernel is a single dense GEMM out = x @ WF. Flatten x to (16384, 4096)
rows, shard rows across 8 cores (2048 rows each), replicate WF. All
matmul operands bf16 (PSUM accumulation fp32; rel err ~2e-3 vs the
2e-2 gate).

Per core, single x-resident block (2048 rows = 128 KB/partition bf16):
  - warmup: 8 junk matmuls on a memset tile (no DMA dependency) so the
    PE HAM clock ramps to 8/8 while the first x/W blocks are in flight.
  - x loads m-split into 512-col quarters, laid out [k4][mq][kk][m] so
    each (k-quad, m-quarter) block is one flat 512KB DMA.
  - DMA ring capabilities (measured): gpsimd SWDGE ~210 GB/s, each
    HWDGE ring (sync, scalar) only ~58 GB/s. Quarter 0 consumes x and
    W(oc0) at ~148 GB/s each, so: all x on gpsimd in PE need-order
    with W(oc0) quads 0/3/6 interleaved at their need points; W(oc0)
    quads 1/4/7 on sync, 2/5 on scalar; W(oc>=1) alternates sync/
    scalar (wpool buffer frees pace them k4-aligned); out rides
    gpsimd, which is idle once x drains (~85us).
  - main GEMM: per o-chunk (512 cols, one PSUM bank), 4 m-quarters of
    4 PSUM banks each (4+4 double buffering: quarter q+1's banks were
    drained during quarter q-1, so bank reuse never stalls the PE);
    x-tile stationary, WF streamed as k-quads [128, 4*512].
  - last quarter (oc7,q3) runs mi-outer/k-inner (all data resident) so
    each PSUM bank finishes 32 MMs before the next starts: drain+store
    pipelines per-mi and the tail is two 128x256 copy+DMA halves
    (~1.5us) instead of 4 copies + a 1MB DMA (~8us).
"""

import sys

if "/opt/trn_rl_repo" not in sys.path:
    sys.path.insert(0, "/opt/trn_rl_repo")

import numpy as np
import ml_dtypes

import concourse.bass as bass
import concourse.mybir as mybir
import concourse.tile as tile

N_CORES = 8
D = 4096
RANK = 16
ROWS_TOTAL = 4 * 4096          # 16384
M = ROWS_TOTAL // N_CORES      # 2048 rows per core
P = 128
KT = D // P                    # 32 k-tiles
OC = 512                       # o-chunk width (one PSUM bank)
N_OC = D // OC                 # 8
MT = M // P                    # 16 m-tiles
MQ = 4                         # m-tiles per quarter (PSUM banks)
NQ = MT // MQ                  # 4 quarters
KQ = 4                         # k-tiles per W quad DMA
N_KQ = KT // KQ                # 8

F32 = mybir.dt.float32
BF16 = mybir.dt.bfloat16
BF16_NP = ml_dtypes.bfloat16

N_WARMUP = 8


def split_wide_waits(nc, max_waits=1):
    """walrus in this container rejects >1 sync wait per instruction;
    move excess waits onto preceding same-engine NoOps."""
    n_split = 0
    for f in nc.m.functions:
        for bb in f.blocks:
            new_insts = []
            for inst in bb.instructions:
                si = getattr(inst, "sync_info", None)
                if si is not None and si.on_wait and len(si.on_wait) > max_waits:
                    waits = list(si.on_wait)
                    keep = waits[-max_waits:]
                    extra = waits[:-max_waits]
                    for i in range(0, len(extra), max_waits):
                        chunk = extra[i:i + max_waits]
                        nop = mybir.InstNoOp(
                            name=f"{inst.name}_wsplit{i}",
                            sync_info=mybir.SyncInfo(on_wait=chunk, on_update=[]),
                            bass_nofuse=True,
                            engine=inst.engine,
                        )
                        new_insts.append(nop)
                        n_split += 1
                    si.on_wait = keep
                new_insts.append(inst)
            bb.instructions[:] = new_insts
    return n_split


def build_program():
    nc = bass.Bass()
    xt = nc.declare_dram_parameter("xt", [D, M], BF16, isOutput=False)
    wf = nc.declare_dram_parameter("wf", [D, D], BF16, isOutput=False)
    out = nc.declare_dram_parameter("out", [M, D], F32, isOutput=True)

    with tile.TileContext(nc) as tc:
        with (
            tc.tile_pool(name="xpool", bufs=1) as xpool,
            tc.tile_pool(name="wpool", bufs=9) as wpool,
            tc.tile_pool(name="opool", bufs=4) as opool,
            tc.tile_pool(name="cpool", bufs=1) as cpool,
            tc.tile_pool(name="ppool", bufs=8, space="PSUM") as ppool,
        ):
            # HAM warmup on a memset tile: no DMA dependency, so the PE
            # is busy (and ramping to 8/8) from ~t0 while x/W stream in.
            jt = cpool.tile([P, OC], BF16, tag="junk_src")
            nc.vector.memset(jt[:], 0.0)
            junk = ppool.tile([P, OC], F32, tag="acc", name="junk")
            for i in range(N_WARMUP):
                nc.tensor.matmul(
                    junk[:],
                    jt[:, :P],
                    jt[:],
                    start=(i == 0),
                    stop=(i == N_WARMUP - 1),
                )

            # x fully resident, laid out [k4][mq][kk][m] so each
            # k-quad x m-quarter block is flat-contiguous: one 512KB DMA
            # per block (32 total), arriving at exactly the granularity
            # the PE consumes (16 main MMs per block).
            xall = xpool.tile([P, KT * M], BF16, tag="x")

            def xsl(k, c0, cw):
                k4, kk = divmod(k, KQ)
                mq, d = divmod(c0, OC)
                assert d + cw <= OC
                base = ((k4 * NQ + mq) * KQ + kk) * OC + d
                return xall[:, base: base + cw]

            def x_load(mq, k4, eng, kk0=0, nkk=KQ):
                base = ((k4 * NQ + mq) * KQ + kk0) * OC
                eng.dma_start(
                    xall[:, base: base + nkk * OC].rearrange(
                        "p (b c) -> p b c", b=nkk),
                    xt[(k4 * KQ + kk0) * P:(k4 * KQ + kk0 + nkk) * P,
                       mq * OC:(mq + 1) * OC].rearrange(
                           "(b p) c -> p b c", p=P),
                )

            def w_dma(wtile, oc, k4, eng, kk0=0, nkk=KQ):
                eng.dma_start(
                    wtile[:, kk0 * OC:(kk0 + nkk) * OC].rearrange(
                        "p (b c) -> p b c", b=nkk),
                    wf[(k4 * KQ + kk0) * P:(k4 * KQ + kk0 + nkk) * P,
                       oc * OC:(oc + 1) * OC].rearrange(
                           "(b p) c -> p b c", p=P),
                )

            def w_quad(oc, k4, eng=None):
                wtile = wpool.tile([P, KQ * OC], BF16, tag="wt")
                w_dma(wtile, oc, k4, eng or nc.sync)
                return wtile

            # Ring capabilities (measured): gpsimd SWDGE ~210 GB/s;
            # each HWDGE ring (sync, scalar) only ~58 GB/s. Quarter 0
            # needs x at 148 GB/s + W(oc0) at 148 GB/s, so: all x on
            # gpsimd in need order, W(oc0) split 3 ways with quads
            # 0/3/6 interleaved into the gpsimd stream at their need
            # points; later W alternates sync/scalar; out rides gpsimd
            # (idle once x drains).
            # buffer allocation in k4 order (so W(oc1,k4) later waits on
            # the free of wtiles0[k4-1] — perfectly pipelined), DMAs
            # issued in per-ring need order.
            wtiles0 = [wpool.tile([P, KQ * OC], BF16, tag="wt",
                                  name=f"wt0_{k4}")
                       for k4 in range(N_KQ)]
            W0G = {0, 3, 6}
            # first quads on the slow HWDGE rings go in halves so their
            # first-half sems fire ~4us sooner
            for kk0 in (0, 2):
                w_dma(wtiles0[1], 0, 1, nc.sync, kk0=kk0, nkk=2)
            for k4 in (4, 7):
                w_dma(wtiles0[k4], 0, k4, nc.sync)
            for kk0 in (0, 2):
                w_dma(wtiles0[2], 0, 2, nc.scalar, kk0=kk0, nkk=2)
            w_dma(wtiles0[5], 0, 5, nc.scalar)
            # gpsimd: kk-granular interleave of the very first x block
            # and W(oc0,0) so the PE's first matmuls unlock ~8us sooner
            # (and HAM ramps off real work instead of stalling cold)
            for kk in range(KQ):
                x_load(0, 0, nc.gpsimd, kk0=kk, nkk=1)
                w_dma(wtiles0[0], 0, 0, nc.gpsimd, kk0=kk, nkk=1)
            for half in (0, 1):
                x_load(0, 1, nc.gpsimd, kk0=2 * half, nkk=2)
            for k4 in range(2, N_KQ):
                x_load(0, k4, nc.gpsimd)
                if k4 in W0G:
                    w_dma(wtiles0[k4], 0, k4, nc.gpsimd)
            for mq in range(1, NQ):
                for k4 in range(N_KQ):
                    x_load(mq, k4, nc.gpsimd)

            def quarter_mms(wtiles, q, psq):
                for k4 in range(N_KQ):
                    for kk in range(KQ):
                        k = KQ * k4 + kk
                        for mi in range(MQ):
                            mt = q * MQ + mi
                            nc.tensor.matmul(
                                psq[mi][:],
                                xsl(k, mt * P, P),
                                wtiles[k4][:, kk * OC:(kk + 1) * OC],
                                start=(k == 0),
                                stop=(k == KT - 1),
                            )

            def out_dma(eng, oc, q, ot, mi=None):
                if mi is None:
                    eng.dma_start(
                        out[q * MQ * P:(q + 1) * MQ * P,
                            oc * OC:(oc + 1) * OC].rearrange(
                                "(b p) c -> p b c", p=P),
                        ot.rearrange("p (b c) -> p b c", b=MQ),
                    )
                else:
                    mt = q * MQ + mi
                    eng.dma_start(
                        out[mt * P:(mt + 1) * P, oc * OC:(oc + 1) * OC],
                        ot[:, mi * OC:(mi + 1) * OC],
                    )

            for oc in range(N_OC):
                wtiles = wtiles0 if oc == 0 else [
                    w_quad(oc, k4, nc.sync if k4 % 2 == 0 else nc.scalar)
                    for k4 in range(N_KQ)
                ]
                for q in range(NQ):
                    psq = [
                        ppool.tile([P, OC], F32, tag="acc",
                                   name=f"ps_{oc}_{q}_{mi}")
                        for mi in range(MQ)
                    ]
                    last = (oc == N_OC - 1 and q == NQ - 1)
                    ot = opool.tile([P, MQ * OC], F32, tag="ot")
                    if not last:
                        quarter_mms(wtiles, q, psq)
                        for mi in range(MQ):
                            nc.vector.tensor_copy(
                                ot[:, mi * OC:(mi + 1) * OC], psq[mi][:]
                            )
                        # out rides the gpsimd ring: x occupies it only
                        # for the first ~85us and HWDGE stays free for W
                        out_dma(nc.gpsimd, oc, q, ot)
                    else:
                        # mi-outer so each PSUM bank completes its 32-MM
                        # accumulation early; drain+store pipeline per-mi
                        # cuts the kernel tail to one copy + one 256KB DMA.
                        for mi in range(MQ):
                            mt = q * MQ + mi
                            for k4 in range(N_KQ):
                                for kk in range(KQ):
                                    k = KQ * k4 + kk
                                    nc.tensor.matmul(
                                        psq[mi][:],
                                        xsl(k, mt * P, P),
                                        wtiles[k4][:, kk * OC:(kk + 1) * OC],
                                        start=(k == 0),
                                        stop=(k == KT - 1),
                                    )
                            if mi < MQ - 1:
                                nc.vector.tensor_copy(
                                    ot[:, mi * OC:(mi + 1) * OC], psq[mi][:]
                                )
                                seng = nc.gpsimd if mi % 2 == 0 else nc.scalar
                                out_dma(seng, oc, q, ot, mi=mi)
                            else:
                                # split the very last drain 4 ways so the
                                # kernel tail is one 128x128 copy + 64KB DMA
                                mt = q * MQ + mi
                                H = OC // 4
                                tengs = [nc.sync, nc.scalar,
                                         nc.sync, nc.scalar]
                                for h in range(4):
                                    nc.vector.tensor_copy(
                                        ot[:, mi * OC + h * H:
                                           mi * OC + (h + 1) * H],
                                        psq[mi][:, h * H:(h + 1) * H],
                                    )
                                    tengs[h].dma_start(
                                        out[mt * P:(mt + 1) * P,
                                            oc * OC + h * H:
                                            oc * OC + (h + 1) * H],
                                        ot[:, mi * OC + h * H:
                                           mi * OC + (h + 1) * H],
                                    )

    split_wide_waits(nc)
    return nc


_NC_CACHE = [None]


def kernel(x, weight, lora_A, lora_B):
    from concourse.bass_utils import run_bass_kernel_spmd

    x = np.asarray(x, dtype=np.float32)
    weight = np.asarray(weight, dtype=np.float32)
    lora_A = np.asarray(lora_A, dtype=np.float32)
    lora_B = np.asarray(lora_B, dtype=np.float32)

    x2 = x.reshape(ROWS_TOTAL, D)
    # fold LoRA into the weight: out = x @ (W.T + 2*B@A), exact rewrite
    wf = np.ascontiguousarray(
        weight.T + 2.0 * (lora_B @ lora_A)
    ).astype(BF16_NP)

    in_maps = []
    for c in range(N_CORES):
        xt_c = np.ascontiguousarray(
            x2[c * M:(c + 1) * M].T
        ).astype(BF16_NP)
        in_maps.append({"xt": xt_c, "wf": wf})

    if _NC_CACHE[0] is None:
        _NC_CACHE[0] = build_program()
    nc = _NC_CACHE[0]

    res = run_bass_kernel_spmd(nc, in_maps, list(range(N_CORES)))
    out = np.concatenate(
        [res.results[c]["out"] for c in range(N_CORES)], axis=0
    )
    return out.reshape(x.shape)


# revision 21
# speedup vs baseline: 1.2686x; 1.0009x over previous
"""LoRA layer kernel for Trainium2, 8-core data-parallel.

out = x @ W.T + 2.0 * ((x @ B) @ A)
  x: (4, 4096, 4096) f32, W: (4096, 4096), A: (16, 4096), B: (4096, 16)

Strategy: fold the LoRA term into the weight on the host
(WF = W.T + 2*B@A, exact same math, 0.5 GFLOP of numpy), so the device
kernel is a single dense GEMM out = x @ WF. Flatten x to (16384, 4096)
rows, shard rows across 8 cores (2048 rows each), replicate WF. All
matmul operands bf16 (PSUM accumulation fp32; rel err ~2e-3 vs the
2e-2 gate).

Per core, single x-resident block (2048 rows = 128 KB/partition bf16):
  - warmup: 8 junk matmuls on a memset tile (no DMA dependency) so the
    PE HAM clock ramps to 8/8 while the first x/W blocks are in flight.
  - x loads m-split into 512-col quarters, laid out [k4][mq][kk][m] so
    each (k-quad, m-quarter) block is one flat 512KB DMA.
  - DMA ring capabilities (measured): gpsimd SWDGE ~210 GB/s, each
    HWDGE ring (sync, scalar) only ~58 GB/s. Quarter 0 consumes x and
    W(oc0) at ~148 GB/s each, so: all x on gpsimd in PE need-order
    with W(oc0) quads 0/3/6 interleaved at their need points; W(oc0)
    quads 1/4/7 on sync, 2/5 on scalar; W(oc>=1) alternates sync/
    scalar (wpool buffer frees pace them k4-aligned); out rides
    gpsimd, which is idle once x drains (~85us).
  - main GEMM: per o-chunk (512 cols, one PSUM bank), 4 m-quarters of
    4 PSUM banks each (4+4 double buffering: quarter q+1's banks were
    drained during quarter q-1, so bank reuse never stalls the PE);
    x-tile stationary, WF streamed as k-quads [128, 4*512].
  - last quarter (oc7,q3) runs mi-outer/k-inner (all data resident) so
    each PSUM bank finishes 32 MMs before the next starts: drain+store
    pipelines per-mi and the tail is two 128x256 copy+DMA halves
    (~1.5us) instead of 4 copies + a 1MB DMA (~8us).
"""

import sys

if "/opt/trn_rl_repo" not in sys.path:
    sys.path.insert(0, "/opt/trn_rl_repo")

import numpy as np
import ml_dtypes

import concourse.bass as bass
import concourse.mybir as mybir
import concourse.tile as tile

N_CORES = 8
D = 4096
RANK = 16
ROWS_TOTAL = 4 * 4096          # 16384
M = ROWS_TOTAL // N_CORES      # 2048 rows per core
P = 128
KT = D // P                    # 32 k-tiles
OC = 512                       # o-chunk width (one PSUM bank)
N_OC = D // OC                 # 8
MT = M // P                    # 16 m-tiles
MQ = 4                         # m-tiles per quarter (PSUM banks)
NQ = MT // MQ                  # 4 quarters
KQ = 4                         # k-tiles per W quad DMA
N_KQ = KT // KQ                # 8

F32 = mybir.dt.float32
BF16 = mybir.dt.bfloat16
BF16_NP = ml_dtypes.bfloat16

N_WARMUP = 14   # junk MMs bridge memset (~8.4us) to first data (~14.5us)


def split_wide_waits(nc, max_waits=1):
    """walrus in this container rejects >1 sync wait per instruction;
    move excess waits onto preceding same-engine NoOps."""
    n_split = 0
    for f in nc.m.functions:
        for bb in f.blocks:
            new_insts = []
            for inst in bb.instructions:
                si = getattr(inst, "sync_info", None)
                if si is not None and si.on_wait and len(si.on_wait) > max_waits:
                    waits = list(si.on_wait)
                    keep = waits[-max_waits:]
                    extra = waits[:-max_waits]
                    for i in range(0, len(extra), max_waits):
                        chunk = extra[i:i + max_waits]
                        nop = mybir.InstNoOp(
                            name=f"{inst.name}_wsplit{i}",
                            sync_info=mybir.SyncInfo(on_wait=chunk, on_update=[]),
                            bass_nofuse=True,
                            engine=inst.engine,
                        )
                        new_insts.append(nop)
                        n_split += 1
                    si.on_wait = keep
                new_insts.append(inst)
            bb.instructions[:] = new_insts
    return n_split


def build_program():
    nc = bass.Bass()
    xt = nc.declare_dram_parameter("xt", [D, M], BF16, isOutput=False)
    wf = nc.declare_dram_parameter("wf", [D, D], BF16, isOutput=False)
    out = nc.declare_dram_parameter("out", [M, D], F32, isOutput=True)

    with tile.TileContext(nc) as tc:
        with (
            tc.tile_pool(name="xpool", bufs=1) as xpool,
            tc.tile_pool(name="wpool", bufs=9) as wpool,
            tc.tile_pool(name="opool", bufs=4) as opool,
            tc.tile_pool(name="cpool", bufs=1) as cpool,
            tc.tile_pool(name="ppool", bufs=8, space="PSUM") as ppool,
        ):
            # HAM warmup on a memset tile: no DMA dependency, so the PE
            # is busy (and ramping to 8/8) from ~t0 while x/W stream in.
            jt = cpool.tile([P, OC], BF16, tag="junk_src")
            nc.vector.memset(jt[:], 0.0)
            junk = ppool.tile([P, OC], F32, tag="acc", name="junk")
            for i in range(N_WARMUP):
                nc.tensor.matmul(
                    junk[:],
                    jt[:, :P],
                    jt[:],
                    start=(i == 0),
                    stop=(i == N_WARMUP - 1),
                )

            # x fully resident, laid out [k4][mq][kk][m] so each
            # k-quad x m-quarter block is flat-contiguous: one 512KB DMA
            # per block (32 total), arriving at exactly the granularity
            # the PE consumes (16 main MMs per block).
            xall = xpool.tile([P, KT * M], BF16, tag="x")

            def xsl(k, c0, cw):
                k4, kk = divmod(k, KQ)
                mq, d = divmod(c0, OC)
                assert d + cw <= OC
                base = ((k4 * NQ + mq) * KQ + kk) * OC + d
                return xall[:, base: base + cw]

            def x_load(mq, k4, eng, kk0=0, nkk=KQ):
                base = ((k4 * NQ + mq) * KQ + kk0) * OC
                eng.dma_start(
                    xall[:, base: base + nkk * OC].rearrange(
                        "p (b c) -> p b c", b=nkk),
                    xt[(k4 * KQ + kk0) * P:(k4 * KQ + kk0 + nkk) * P,
                       mq * OC:(mq + 1) * OC].rearrange(
                           "(b p) c -> p b c", p=P),
                )

            def w_dma(wtile, oc, k4, eng, kk0=0, nkk=KQ):
                eng.dma_start(
                    wtile[:, kk0 * OC:(kk0 + nkk) * OC].rearrange(
                        "p (b c) -> p b c", b=nkk),
                    wf[(k4 * KQ + kk0) * P:(k4 * KQ + kk0 + nkk) * P,
                       oc * OC:(oc + 1) * OC].rearrange(
                           "(b p) c -> p b c", p=P),
                )

            def w_quad(oc, k4, eng=None):
                wtile = wpool.tile([P, KQ * OC], BF16, tag="wt")
                w_dma(wtile, oc, k4, eng or nc.sync)
                return wtile

            # Ring capabilities (measured): gpsimd SWDGE ~210 GB/s;
            # each HWDGE ring (sync, scalar) only ~58 GB/s. Quarter 0
            # needs x at 148 GB/s + W(oc0) at 148 GB/s, so: all x on
            # gpsimd in need order, W(oc0) split 3 ways with quads
            # 0/3/6 interleaved into the gpsimd stream at their need
            # points; later W alternates sync/scalar; out rides gpsimd
            # (idle once x drains).
            # buffer allocation in k4 order (so W(oc1,k4) later waits on
            # the free of wtiles0[k4-1] — perfectly pipelined), DMAs
            # issued in per-ring need order.
            wtiles0 = [wpool.tile([P, KQ * OC], BF16, tag="wt",
                                  name=f"wt0_{k4}")
                       for k4 in range(N_KQ)]
            # Ring start latencies (measured): HWDGE sync/scalar deliver
            # from ~9-10us (burst ~113 GB/s solo, ~70 sustained); the
            # gpsimd SWDGE ring starts ~11.4us then sustains ~210 GB/s.
            # W quads are atomic deps (readers of a multi-write tile
            # wait for its LAST write), so never split them; x00 kk
            # chunks unlock progressively (xall region deps work).
            # sync: W00 first (gates the first real MM, ~14.5us), then
            # W02, W07. scalar: x00 chunks, then W04, W06. gpsimd: the
            # rest of x in need order with W01/W03/W05 interleaved.
            w_dma(wtiles0[0], 0, 0, nc.sync)
            w_dma(wtiles0[2], 0, 2, nc.sync)
            w_dma(wtiles0[7], 0, 7, nc.sync)
            for kk in range(KQ):
                x_load(0, 0, nc.scalar, kk0=kk, nkk=1)
            w_dma(wtiles0[4], 0, 4, nc.scalar)
            w_dma(wtiles0[6], 0, 6, nc.scalar)
            gw = {1: 1, 2: 3, 3: 5}   # after x0k, load W(oc0, gw[k4])
            for k4 in range(1, N_KQ):
                x_load(0, k4, nc.gpsimd)
                if k4 in gw:
                    w_dma(wtiles0[gw[k4]], 0, gw[k4], nc.gpsimd)
            for mq in range(1, NQ):
                for k4 in range(N_KQ):
                    x_load(mq, k4, nc.gpsimd)

            def quarter_mms(wtiles, q, psq):
                for k4 in range(N_KQ):
                    for kk in range(KQ):
                        k = KQ * k4 + kk
                        for mi in range(MQ):
                            mt = q * MQ + mi
                            nc.tensor.matmul(
                                psq[mi][:],
                                xsl(k, mt * P, P),
                                wtiles[k4][:, kk * OC:(kk + 1) * OC],
                                start=(k == 0),
                                stop=(k == KT - 1),
                            )

            def out_dma(eng, oc, q, ot, mi=None):
                if mi is None:
                    eng.dma_start(
                        out[q * MQ * P:(q + 1) * MQ * P,
                            oc * OC:(oc + 1) * OC].rearrange(
                                "(b p) c -> p b c", p=P),
                        ot.rearrange("p (b c) -> p b c", b=MQ),
                    )
                else:
                    mt = q * MQ + mi
                    eng.dma_start(
                        out[mt * P:(mt + 1) * P, oc * OC:(oc + 1) * OC],
                        ot[:, mi * OC:(mi + 1) * OC],
                    )

            for oc in range(N_OC):
                wtiles = wtiles0 if oc == 0 else [
                    w_quad(oc, k4, nc.sync if k4 % 2 == 0 else nc.scalar)
                    for k4 in range(N_KQ)
                ]
                for q in range(NQ):
                    psq = [
                        ppool.tile([P, OC], F32, tag="acc",
                                   name=f"ps_{oc}_{q}_{mi}")
                        for mi in range(MQ)
                    ]
                    last = (oc == N_OC - 1 and q == NQ - 1)
                    ot = opool.tile([P, MQ * OC], F32, tag="ot")
                    if not last:
                        quarter_mms(wtiles, q, psq)
                        for mi in range(MQ):
                            nc.vector.tensor_copy(
                                ot[:, mi * OC:(mi + 1) * OC], psq[mi][:]
                            )
                        # out rides the gpsimd ring: x occupies it only
                        # for the first ~85us and HWDGE stays free for W
                        out_dma(nc.gpsimd, oc, q, ot)
                    else:
                        # mi-outer so each PSUM bank completes its 32-MM
                        # accumulation early; drain+store pipeline per-mi
                        # cuts the kernel tail to one copy + one 256KB DMA.
                        for mi in range(MQ):
                            mt = q * MQ + mi
                            for k4 in range(N_KQ):
                                for kk in range(KQ):
                                    k = KQ * k4 + kk
                                    nc.tensor.matmul(
                                        psq[mi][:],
                                        xsl(k, mt * P, P),
                                        wtiles[k4][:, kk * OC:(kk + 1) * OC],
                                        start=(k == 0),
                                        stop=(k == KT - 1),
                                    )
                            if mi < MQ - 1:
                                nc.vector.tensor_copy(
                                    ot[:, mi * OC:(mi + 1) * OC], psq[mi][:]
                                )
                                seng = nc.gpsimd if mi % 2 == 0 else nc.scalar
                                out_dma(seng, oc, q, ot, mi=mi)
                            else:
                                # split the very last drain 4 ways so the
                                # kernel tail is one 128x128 copy + 64KB DMA
                                mt = q * MQ + mi
                                H = OC // 4
                                tengs = [nc.sync, nc.scalar,
                                         nc.sync, nc.scalar]
                                for h in range(4):
                                    nc.vector.tensor_copy(
                                        ot[:, mi * OC + h * H:
                                           mi * OC + (h + 1) * H],
                                        psq[mi][:, h * H:(h + 1) * H],
                                    )
                                    tengs[h].dma_start(
                                        out[mt * P:(mt + 1) * P,
                                            oc * OC + h * H:
                                            oc * OC + (h + 1) * H],
                                        ot[:, mi * OC + h * H:
                                           mi * OC + (h + 1) * H],
                                    )

    split_wide_waits(nc)
    return nc


_NC_CACHE = [None]


def kernel(x, weight, lora_A, lora_B):
    from concourse.bass_utils import run_bass_kernel_spmd

    x = np.asarray(x, dtype=np.float32)
    weight = np.asarray(weight, dtype=np.float32)
    lora_A = np.asarray(lora_A, dtype=np.float32)
    lora_B = np.asarray(lora_B, dtype=np.float32)

    x2 = x.reshape(ROWS_TOTAL, D)
    # fold LoRA into the weight: out = x @ (W.T + 2*B@A), exact rewrite
    wf = np.ascontiguousarray(
        weight.T + 2.0 * (lora_B @ lora_A)
    ).astype(BF16_NP)

    in_maps = []
    for c in range(N_CORES):
        xt_c = np.ascontiguousarray(
            x2[c * M:(c + 1) * M].T
        ).astype(BF16_NP)
        in_maps.append({"xt": xt_c, "wf": wf})

    if _NC_CACHE[0] is None:
        _NC_CACHE[0] = build_program()
    nc = _NC_CACHE[0]

    res = run_bass_kernel_spmd(nc, in_maps, list(range(N_CORES)))
    out = np.concatenate(
        [res.results[c]["out"] for c in range(N_CORES)], axis=0
    )
    return out.reshape(x.shape)
